# revision 36
# baseline (speedup 1.0000x reference)
"""Grouped (MoE-style) linear on 8 trn2 NeuronCores.

out[t] = hidden_states[t] @ weight[g(t)], where token t belongs to group g iff
offsets[g-1] <= t < offsets[g] (searchsorted right semantics; tokens at or past
offsets[-1] get zero output).

Strategy: expert-parallel. Core g owns weight[g] and the contiguous token run
of group g; routing is host-side (offsets are host data). Each core runs an
identical Bass program: [ntb*128, 1024] x [1024, 1024] matmul in bf16
(1 cycle/row on the PE, same rate as fp32r, but half the DMA traffic; ~4e-3
max rel err for this distribution, incl. bf16 output rounding).

Measured HW constraints this schedule is built around (from NTFF traces):
  - ~6.5us fixed NEFF preamble before any user instruction on every queue.
  - DMA issue -> first byte ~2.4us; all queues share ~345 GB/s of HBM bw.
  - PE p-state: ~0.65 GHz cold, 2.4 GHz after ~3us of continuous matmul.
  - A 512-col bf16 matmul is 213ns at full clock; the 256-matmul stream is
    54.6us and is the floor, so everything else must hide under/around it.

Schedule per core (ntb=16 for the 2048-token groups; exec ~73us vs the
81.4us fp32r baseline):
  - Every tile is written by exactly ONE DMA so consumers wait on exactly
    the bytes they need (a shared tile serializes on its last writer).
  - Pass A (first 3 blocks) X loads are split into k-half tiles and W into
    half-column tiles; each DMA queue's FIRST transfer is a k0-batch
    dependency (x0a sync, x1a gpsimd, Wk0-lo scalar), because under 3-way
    contention a queue's 2nd transfer lands ~1.2us later. Bulk X (blocks
    3..) streams on scalar strictly behind W so it never starves W.
  - 7 warmup matmuls on memset junk tiles ramp the PE p-state during the
    load window, timed to end right as the first deps land (~11us); a gap
    there costs ~1.5us of re-ramp at 1.2GHz.
  - Pass A runs k-major (lo-half-major within each k batch) so compute
    pace stays behind W half arrival; later blocks run block-major so
    stores spread evenly instead of bunching per pass.
  - PSUM->SBUF copies downcast to bf16 (scalar lo / vector hi halves);
    stores alternate sync/gpsimd. The first block-major block and the
    final block run as two half-column groups in their own 1-bank PSUM
    tiles: the former delays the full-PSUM-pool reuse past pass A's
    copies, the latter lets the hi half store while the lo half computes,
    leaving a ~2.7us drain tail (copy pair + two 64KB stores).
"""
import numpy as np
import ml_dtypes

import concourse.bass as bass
import concourse.tile as tile
from concourse import bacc, mybir
from concourse.bass_utils import run_bass_kernel_spmd

GROUPS = 8
TOKENS = 16384
IN_F = 1024
OUT_F = 1024
KCH = IN_F // 128  # contraction chunks
BF16 = ml_dtypes.bfloat16
NWARM = 7          # PE p-state warmup matmuls
XBUFS = 24         # X tile ring (full preload for ntb<=24)
LOOK = 8           # bulk-X issue lookahead (blocks), < XBUFS - PSUM window


def build(ntb: int) -> bass.Bass:
    """One core's program: ntb 128-token blocks through a 1024x1024 expert."""
    bf16 = mybir.dt.bfloat16
    f32 = mybir.dt.float32
    nc = bacc.Bacc()
    x_d = nc.dram_tensor("x", [ntb, 128, KCH, 128], bf16, kind="ExternalInput")
    w_d = nc.dram_tensor("w", [KCH, 128, OUT_F], bf16, kind="ExternalInput")
    out_d = nc.dram_tensor("out", [ntb * 128, OUT_F], bf16, kind="ExternalOutput")

    na = min(3, ntb)  # k-major pass-A width (3 x 2 PSUM banks; 2 banks kept
                      # for the final block's staggered half-column groups)

    with tile.TileContext(nc) as tc:
        with (
            tc.tile_pool(name="wp", bufs=2 * KCH) as wp,
            tc.tile_pool(name="zp", bufs=1) as zp,
            tc.tile_pool(name="xhp", bufs=2 * na) as xhp,
            tc.tile_pool(name="xp", bufs=min(ntb, XBUFS)) as xp,
            tc.tile_pool(name="op", bufs=6) as op,
            tc.tile_pool(name="ps", bufs=3, space="PSUM") as psp,
            tc.tile_pool(name="psq", bufs=2, space="PSUM") as psqp,
        ):
            # junk tiles for PE p-state warmup; memset on vector so the
            # sync/gpsimd/scalar DMA queues start issuing immediately
            zs = zp.tile([128, 128], bf16, tag="zs")
            zm = zp.tile([128, 512], bf16, tag="zm")
            nc.vector.memset(zs[:], 0.0)
            nc.vector.memset(zm[:], 0.0)

            xts = [None] * ntb   # full-block tiles [128, KCH, 128]
            xhs = {}             # pass-A half tiles: (b, half) -> [128, KCH//2, 128]

            def issue_x(b, q):
                xt = xp.tile([128, KCH, 128], bf16, tag="x", name="xt")
                q.dma_start(out=xt[:], in_=x_d[b])
                xts[b] = xt

            def issue_xh(b, h, q):
                xh = xhp.tile([128, KCH // 2, 128], bf16, tag="xh", name="xh")
                q.dma_start(out=xh[:], in_=x_d[b, :, h * (KCH // 2):(h + 1) * (KCH // 2), :])
                xhs[(b, h)] = xh

            # pass-A X blocks in k-half tiles (exact deps). Every queue's
            # FIRST DMA is a k0-batch dependency (under 3-way contention a
            # queue's 2nd transfer lands ~1.2us later): x0a on sync, x1a on
            # gpsimd, W k0-lo then x2a on scalar.
            wts = [[None, None] for _ in range(KCH)]

            def issue_w(k, h, q=None):
                wt = wp.tile([128, OUT_F // 2], bf16, tag="w", name="wt")
                (q or nc.scalar).dma_start(
                    out=wt[:], in_=w_d[k, :, h * 512:(h + 1) * 512])
                wts[k][h] = wt

            issue_xh(0, 0, nc.sync)
            if na > 1:
                issue_xh(1, 0, nc.gpsimd)
            issue_w(0, 0)
            if na > 2:
                issue_xh(2, 0, nc.scalar)
            issue_w(0, 1)
            for k in range(1, KCH):
                issue_w(k, 0)
                issue_w(k, 1)

            for b in range(na):
                issue_xh(b, 1, [nc.sync, nc.gpsimd][b % 2])

            # first slice of bulk X behind W on scalar
            for b in range(na, min(na + LOOK, ntb)):
                issue_x(b, nc.scalar)

            def xsrc(b, k):
                if (b, 0) in xhs:
                    h, kk = divmod(k, KCH // 2)
                    return xhs[(b, h)][:, kk, :]
                return xts[b][:, k, :]

            def mm(ps, b, k):
                for nb in range(2):
                    nc.tensor.matmul(
                        ps[:, nb * 512:(nb + 1) * 512],
                        xsrc(b, k),
                        wts[k][nb][:],
                        start=(k == 0),
                        stop=(k == KCH - 1),
                    )

            def emit_half_block(b, ot_last=None):
                """Final-ish block via two half-column groups in 1-bank PSUM
                tiles; hi finishes (and stores) before lo computes."""
                r = b * 128
                ph = psqp.tile([128, 512], f32, tag="psq", name="psq")
                for k in range(KCH):
                    nc.tensor.matmul(ph[:], xsrc(b, k), wts[k][1][:],
                                     start=(k == 0), stop=(k == KCH - 1))
                ot = op.tile([128, OUT_F], bf16, tag="ot", name="ot")
                nc.vector.tensor_copy(ot[:, 512:1024], ph[:])
                nc.gpsimd.dma_start(out=out_d[r:r + 128, 512:1024],
                                    in_=ot[:, 512:1024])
                pl = psqp.tile([128, 512], f32, tag="psq", name="psq")
                for k in range(KCH):
                    nc.tensor.matmul(pl[:], xsrc(b, k), wts[k][0][:],
                                     start=(k == 0), stop=(k == KCH - 1))
                if b == ntb - 1:
                    # one copy (split copies serialize on the copy-ordering
                    # sem anyway), then two parallel 64KB quarter stores
                    nc.scalar.copy(ot[:, 0:512], pl[:])
                    nc.sync.dma_start(out=out_d[r:r + 128, 0:256],
                                      in_=ot[:, 0:256])
                    nc.gpsimd.dma_start(out=out_d[r:r + 128, 256:512],
                                        in_=ot[:, 256:512])
                else:
                    nc.scalar.copy(ot[:, 0:512], pl[:])
                    nc.sync.dma_start(out=out_d[r:r + 128, 0:512],
                                      in_=ot[:, 0:512])

            def emit_out(ps, b):
                ot = op.tile([128, OUT_F], bf16, tag="ot", name="ot")
                nc.scalar.copy(ot[:, 0:512], ps[:, 0:512])
                nc.vector.tensor_copy(ot[:, 512:1024], ps[:, 512:1024])
                r = b * 128
                if b == ntb - 1:
                    nc.sync.dma_start(out=out_d[r:r + 128, 0:512],
                                      in_=ot[:, 0:512])
                    nc.gpsimd.dma_start(out=out_d[r:r + 128, 512:1024],
                                        in_=ot[:, 512:1024])
                else:
                    q = nc.sync if b % 2 == 0 else nc.gpsimd
                    q.dma_start(out=out_d[r:r + 128, :], in_=ot[:])

            # --- pass A: k-major over the first na blocks ---
            pss = [psp.tile([128, OUT_F], f32, tag="ps", name="ps")
                   for _ in range(na)]
            for _ in range(NWARM):
                # p-state warmup; start=True resets the bank so only real
                # work's accumulation counts.
                nc.tensor.matmul(pss[0][:, 0:512], zs[:], zm[:],
                                 start=True, stop=True, skip_group_check=True)
            for k in range(KCH):
                # lo-half-major: the first matmuls of each k batch need only
                # the lo W half, which lands ~0.65us before the hi half
                for nb in range(2):
                    for j in range(na):
                        nc.tensor.matmul(
                            pss[j][:, nb * 512:(nb + 1) * 512],
                            xsrc(j, k),
                            wts[k][nb][:],
                            start=(k == 0),
                            stop=(k == KCH - 1),
                        )
            for j in range(na):
                emit_out(pss[j], j)

            # --- remaining blocks: block-major, bulk-X lookahead on scalar ---
            for b in range(na, ntb):
                if b + LOOK < ntb:
                    issue_x(b + LOOK, nc.scalar)
                if b == na or b == ntb - 1:
                    # psq path for the first block-major block (so the full
                    # ps pool's first reuse waits one block longer than pass
                    # A's copies need) and for the final block (short tail).
                    emit_half_block(b)
                else:
                    ps = psp.tile([128, OUT_F], f32, tag="ps", name="ps")
                    for k in range(KCH):
                        mm(ps, b, k)
                    emit_out(ps, b)
    nc.compile()
    return nc


def _pack_core(x_slice: np.ndarray, ntb: int):
    n = x_slice.shape[0]
    xp = np.zeros((ntb * 128, IN_F), dtype=BF16)
    xp[:n] = x_slice
    # [tb, tok, k, p] -> [tb, p, k, tok]
    return np.ascontiguousarray(xp.reshape(ntb, 128, KCH, 128).transpose(0, 3, 2, 1))


def kernel(hidden_states: np.ndarray, weight: np.ndarray, offsets: np.ndarray,
           _trace: bool = False):
    hs = np.asarray(hidden_states, dtype=np.float32).astype(BF16)
    w = np.asarray(weight, dtype=np.float32).astype(BF16)
    off = np.asarray(offsets).astype(np.int64)

    ends = np.clip(off, 0, TOKENS)
    starts = np.concatenate(([0], ends[:-1]))
    starts = np.minimum(starts, ends)
    ns = ends - starts

    ntb = max(1, int(-(-ns.max() // 128)))
    nc = build(ntb)

    in_maps = []
    for g in range(GROUPS):
        in_maps.append({
            "x": _pack_core(hs[starts[g]:ends[g]], ntb),
            "w": np.ascontiguousarray(w[g].reshape(KCH, 128, OUT_F)),
        })

    res = run_bass_kernel_spmd(nc, in_maps, list(range(GROUPS)), trace=_trace)

    out = np.zeros((TOKENS, OUT_F), dtype=np.float32)
    for g in range(GROUPS):
        if ns[g] > 0:
            out[starts[g]:ends[g]] = res.results[g]["out"][:ns[g]].astype(np.float32)
    if _trace:
        return out, res
    return out


# revision 37
# speedup vs baseline: 1.0154x; 1.0154x over previous
"""Grouped (MoE-style) linear on 8 trn2 NeuronCores.

out[t] = hidden_states[t] @ weight[g(t)], where token t belongs to group g iff
offsets[g-1] <= t < offsets[g] (searchsorted right semantics; tokens at or past
offsets[-1] get zero output).

Strategy: expert-parallel. Core g owns weight[g] and the contiguous token run
of group g; routing is host-side (offsets are host data). Each core runs an
identical Bass program: [ntb*128, 1024] x [1024, 1024] matmul in bf16
(1 cycle/row on the PE, same rate as fp32r, but half the DMA traffic; ~4e-3
max rel err for this distribution, incl. bf16 output rounding).

Measured HW constraints this schedule is built around (from NTFF traces):
  - ~6.5us fixed NEFF preamble before any user instruction on every queue.
  - DMA issue -> first byte ~2.4us; all queues share ~345 GB/s of HBM bw.
  - PE p-state: ~0.65 GHz cold, 2.4 GHz after ~3us of continuous matmul.
  - A 512-col bf16 matmul is 213ns at full clock; the 256-matmul stream is
    54.6us and is the floor, so everything else must hide under/around it.

Schedule per core (ntb=16 for the 2048-token groups; exec ~73us vs the
81.4us fp32r baseline):
  - Every tile is written by exactly ONE DMA so consumers wait on exactly
    the bytes they need (a shared tile serializes on its last writer).
  - Pass A (first 3 blocks) X loads are split into k-half tiles and W into
    half-column tiles; each DMA queue's FIRST transfer is a k0-batch
    dependency (x0a sync, x1a gpsimd, Wk0-lo scalar), because under 3-way
    contention a queue's 2nd transfer lands ~1.2us later. Bulk X (blocks
    3..) streams on scalar strictly behind W so it never starves W.
  - 7 warmup matmuls on memset junk tiles ramp the PE p-state during the
    load window, timed to end right as the first deps land (~11us); a gap
    there costs ~1.5us of re-ramp at 1.2GHz.
  - Pass A runs k-major (lo-half-major within each k batch) so compute
    pace stays behind W half arrival; later blocks run block-major so
    stores spread evenly instead of bunching per pass.
  - PSUM->SBUF copies downcast to bf16 (scalar lo / vector hi halves);
    stores alternate sync/gpsimd. The first block-major block and the
    final block run as two half-column groups in their own 1-bank PSUM
    tiles: the former delays the full-PSUM-pool reuse past pass A's
    copies, the latter lets the hi half store while the lo half computes,
    leaving a ~2.7us drain tail (copy pair + two 64KB stores).
"""
import numpy as np
import ml_dtypes

import concourse.bass as bass
import concourse.tile as tile
from concourse import bacc, mybir
from concourse.bass_utils import run_bass_kernel_spmd

GROUPS = 8
TOKENS = 16384
IN_F = 1024
OUT_F = 1024
KCH = IN_F // 128  # contraction chunks
BF16 = ml_dtypes.bfloat16
NWARM = 7          # PE p-state warmup matmuls
XBUFS = 24         # X tile ring (full preload for ntb<=24)
LOOK = 8           # bulk-X issue lookahead (blocks), < XBUFS - PSUM window


def build(ntb: int) -> bass.Bass:
    """One core's program: ntb 128-token blocks through a 1024x1024 expert."""
    bf16 = mybir.dt.bfloat16
    f32 = mybir.dt.float32
    nc = bacc.Bacc()
    x_d = nc.dram_tensor("x", [ntb, 128, KCH, 128], bf16, kind="ExternalInput")
    w_d = nc.dram_tensor("w", [KCH, 128, OUT_F], bf16, kind="ExternalInput")
    out_d = nc.dram_tensor("out", [ntb * 128, OUT_F], bf16, kind="ExternalOutput")

    na = min(3, ntb)  # k-major pass-A width (3 x 2 PSUM banks; 2 banks kept
                      # for the final block's staggered half-column groups)

    with tile.TileContext(nc) as tc:
        with (
            tc.tile_pool(name="wp", bufs=2 * KCH) as wp,
            tc.tile_pool(name="zp", bufs=1) as zp,
            tc.tile_pool(name="xhp", bufs=2 * na) as xhp,
            tc.tile_pool(name="xp", bufs=min(ntb, XBUFS)) as xp,
            tc.tile_pool(name="op", bufs=6) as op,
            tc.tile_pool(name="ps", bufs=3, space="PSUM") as psp,
            tc.tile_pool(name="psq", bufs=2, space="PSUM") as psqp,
        ):
            # junk tiles for PE p-state warmup; memset on vector so the
            # sync/gpsimd/scalar DMA queues start issuing immediately
            zs = zp.tile([128, 128], bf16, tag="zs")
            zm = zp.tile([128, 512], bf16, tag="zm")
            nc.vector.memset(zs[:], 0.0)
            nc.vector.memset(zm[:], 0.0)

            xts = [None] * ntb   # full-block tiles [128, KCH, 128]
            xhs = {}             # pass-A half tiles: (b, half) -> [128, KCH//2, 128]

            def issue_x(b, q):
                xt = xp.tile([128, KCH, 128], bf16, tag="x", name="xt")
                q.dma_start(out=xt[:], in_=x_d[b])
                xts[b] = xt

            def issue_xh(b, h, q):
                xh = xhp.tile([128, KCH // 2, 128], bf16, tag="xh", name="xh")
                q.dma_start(out=xh[:], in_=x_d[b, :, h * (KCH // 2):(h + 1) * (KCH // 2), :])
                xhs[(b, h)] = xh

            # pass-A X blocks in k-half tiles (exact deps). Every queue's
            # FIRST DMA is a k0-batch dependency (under 3-way contention a
            # queue's 2nd transfer lands ~1.2us later): x0a on sync, x1a on
            # gpsimd, W k0-lo then x2a on scalar.
            wts = [[None, None] for _ in range(KCH)]

            def issue_w(k, h, q=None):
                wt = wp.tile([128, OUT_F // 2], bf16, tag="w", name="wt")
                (q or nc.scalar).dma_start(
                    out=wt[:], in_=w_d[k, :, h * 512:(h + 1) * 512])
                wts[k][h] = wt

            issue_xh(0, 0, nc.sync)
            if na > 1:
                issue_xh(1, 0, nc.gpsimd)
            issue_w(0, 0)
            if na > 2:
                issue_xh(2, 0, nc.scalar)
            issue_w(0, 1)
            for k in range(1, KCH):
                issue_w(k, 0)
                issue_w(k, 1)

            for b in range(na):
                issue_xh(b, 1, [nc.sync, nc.gpsimd][b % 2])

            # first slice of bulk X behind W on scalar
            for b in range(na, min(na + LOOK, ntb)):
                issue_x(b, nc.scalar)

            def xsrc(b, k):
                if (b, 0) in xhs:
                    h, kk = divmod(k, KCH // 2)
                    return xhs[(b, h)][:, kk, :]
                return xts[b][:, k, :]

            def mm(ps, b, k):
                for nb in range(2):
                    nc.tensor.matmul(
                        ps[:, nb * 512:(nb + 1) * 512],
                        xsrc(b, k),
                        wts[k][nb][:],
                        start=(k == 0),
                        stop=(k == KCH - 1),
                    )

            def emit_half_block(b, ot_last=None):
                """Final-ish block via two half-column groups in 1-bank PSUM
                tiles; hi finishes (and stores) before lo computes."""
                r = b * 128
                ph = psqp.tile([128, 512], f32, tag="psq", name="psq")
                for k in range(KCH):
                    nc.tensor.matmul(ph[:], xsrc(b, k), wts[k][1][:],
                                     start=(k == 0), stop=(k == KCH - 1))
                ot = op.tile([128, OUT_F], bf16, tag="ot", name="ot")
                nc.vector.tensor_copy(ot[:, 512:1024], ph[:])
                nc.gpsimd.dma_start(out=out_d[r:r + 128, 512:1024],
                                    in_=ot[:, 512:1024])
                pl = psqp.tile([128, 512], f32, tag="psq", name="psq")
                for k in range(KCH):
                    nc.tensor.matmul(pl[:], xsrc(b, k), wts[k][0][:],
                                     start=(k == 0), stop=(k == KCH - 1))
                if b == ntb - 1:
                    nc.scalar.copy(ot[:, 0:256], pl[:, 0:256])
                    nc.vector.tensor_copy(ot[:, 256:512], pl[:, 256:512])
                    nc.sync.dma_start(out=out_d[r:r + 128, 0:256],
                                      in_=ot[:, 0:256])
                    nc.scalar.dma_start(out=out_d[r:r + 128, 256:512],
                                        in_=ot[:, 256:512])
                else:
                    nc.scalar.copy(ot[:, 0:512], pl[:])
                    nc.sync.dma_start(out=out_d[r:r + 128, 0:512],
                                      in_=ot[:, 0:512])

            def emit_out(ps, b):
                ot = op.tile([128, OUT_F], bf16, tag="ot", name="ot")
                nc.scalar.copy(ot[:, 0:512], ps[:, 0:512])
                nc.vector.tensor_copy(ot[:, 512:1024], ps[:, 512:1024])
                r = b * 128
                if b == ntb - 1:
                    nc.sync.dma_start(out=out_d[r:r + 128, 0:512],
                                      in_=ot[:, 0:512])
                    nc.gpsimd.dma_start(out=out_d[r:r + 128, 512:1024],
                                        in_=ot[:, 512:1024])
                else:
                    q = nc.sync if b % 2 == 0 else nc.gpsimd
                    q.dma_start(out=out_d[r:r + 128, :], in_=ot[:])

            # --- pass A: k-major over the first na blocks ---
            pss = [psp.tile([128, OUT_F], f32, tag="ps", name="ps")
                   for _ in range(na)]
            for _ in range(NWARM):
                # p-state warmup; start=True resets the bank so only real
                # work's accumulation counts.
                nc.tensor.matmul(pss[0][:, 0:512], zs[:], zm[:],
                                 start=True, stop=True, skip_group_check=True)
            for k in range(KCH):
                # lo-half-major: the first matmuls of each k batch need only
                # the lo W half, which lands ~0.65us before the hi half
                for nb in range(2):
                    for j in range(na):
                        nc.tensor.matmul(
                            pss[j][:, nb * 512:(nb + 1) * 512],
                            xsrc(j, k),
                            wts[k][nb][:],
                            start=(k == 0),
                            stop=(k == KCH - 1),
                        )
            for j in range(na):
                emit_out(pss[j], j)

            # --- remaining blocks: block-major, bulk-X lookahead on scalar ---
            for b in range(na, ntb):
                if b + LOOK < ntb:
                    issue_x(b + LOOK, nc.scalar)
                if b == na or b == ntb - 1:
                    # psq path for the first block-major block (so the full
                    # ps pool's first reuse waits one block longer than pass
                    # A's copies need) and for the final block (short tail).
                    emit_half_block(b)
                else:
                    ps = psp.tile([128, OUT_F], f32, tag="ps", name="ps")
                    for k in range(KCH):
                        mm(ps, b, k)
                    emit_out(ps, b)
    nc.compile()
    return nc


def _pack_core(x_slice: np.ndarray, ntb: int):
    n = x_slice.shape[0]
    xp = np.zeros((ntb * 128, IN_F), dtype=BF16)
    xp[:n] = x_slice
    # [tb, tok, k, p] -> [tb, p, k, tok]
    return np.ascontiguousarray(xp.reshape(ntb, 128, KCH, 128).transpose(0, 3, 2, 1))


def kernel(hidden_states: np.ndarray, weight: np.ndarray, offsets: np.ndarray,
           _trace: bool = False):
    hs = np.asarray(hidden_states, dtype=np.float32).astype(BF16)
    w = np.asarray(weight, dtype=np.float32).astype(BF16)
    off = np.asarray(offsets).astype(np.int64)

    ends = np.clip(off, 0, TOKENS)
    starts = np.concatenate(([0], ends[:-1]))
    starts = np.minimum(starts, ends)
    ns = ends - starts

    ntb = max(1, int(-(-ns.max() // 128)))
    nc = build(ntb)

    in_maps = []
    for g in range(GROUPS):
        in_maps.append({
            "x": _pack_core(hs[starts[g]:ends[g]], ntb),
            "w": np.ascontiguousarray(w[g].reshape(KCH, 128, OUT_F)),
        })

    res = run_bass_kernel_spmd(nc, in_maps, list(range(GROUPS)), trace=_trace)

    out = np.zeros((TOKENS, OUT_F), dtype=np.float32)
    for g in range(GROUPS):
        if ns[g] > 0:
            out[starts[g]:ends[g]] = res.results[g]["out"][:ns[g]].astype(np.float32)
    if _trace:
        return out, res
    return out
